# revision 3
# baseline (speedup 1.0000x reference)
"""Multi-head cross-attention (B=4, S=2048, D=1024, H=16) on 8 Trainium2 cores.

Sharding: hybrid data/tensor parallel. Core c handles batch b = c//2 and
head-group g = c%2 (8 of the 16 heads, i.e. 512 of the 1024 q/k/v dims).
Each core computes a partial out-projection over its 512 attention dims;
the host sums the two partials per batch.

v2 vs baseline: the S=K.T@Q logits matmuls contract over head_dim=64, so
they are issued as PE row-tiled pairs (tile_position (0,0) and (64,0)) —
the even head streams on array rows 0:63 while the odd head streams on
rows 64:127 concurrently, halving PE time for the logits stage. kT packs
a head pair per tile ([0:64]=even head dims, [64:128]=odd). The ACT
(scalar) engine runs ONLY the exp activations (its 266us is the kernel
floor); all bias/identity/normalize work is on DVE/Pool. PSUM: lg_e +
lg_o + av_e + av_o = 4 tags x 2 banks = 8 banks.

Per-core kernel (fp16 PE fast path):
  K.T = wk_t.T @ mem_t (+bk)      [512, 2048]  packed per head-pair
  V   = mem_t.T @ wv_t            [2048, 512] stored as v_aug [*, 8, 65]
                                  with a ones column per head (denominator)
  Q.T = wq_t.T @ x_t   (+bq)      [512, 2048]
  per head-pair mt, query-half qh, key chunk kc:
    S_e.T[k,q] = K_e @ Q_e.T   (row tile (0,0), K=64)   } concurrent
    S_o.T[k,q] = K_o @ Q_o.T   (row tile (64,0), K=64)  }
    P.T = exp(0.125*S.T + mask_bias)  (ACT, per-key-partition bias)
    AV.T += [V_h|1].T @ P.T    [65, 1024] PSUM accum over kc
  attn.T = AV.T[0:64] * recip(AV.T[64])
  out.T = wo_t.T @ attn.T (+bo_eff on core g=0)  [1024, 2048] partial

bv is folded into bo on the host: out = attn@wo.T + (bo + wo@bv) because
softmax rows sum to 1. The key-padding mask enters as an additive
per-partition bias in the exp activation (exact, and free).
"""

import numpy as np

import concourse.bacc as bacc
import concourse.mybir as mybir
from concourse import tile
from concourse.bass_utils import run_bass_kernel_spmd

F32 = mybir.dt.float32
F16 = mybir.dt.float16
AF = mybir.ActivationFunctionType

B, S, D = 4, 2048, 1024
H, HD = 16, 64
NCORES = 8
NH = 8          # heads per core
OD = NH * HD    # 512 attention dims per core
P = 128
NDC = D // P    # 8 d-chunks
NKC = S // P    # 16 key chunks
NEG = -1.0e30

_cache = {}


def _build():
    from contextlib import ExitStack

    nc = bacc.Bacc(None, target_bir_lowering=False, debug=False)

    x_t = nc.dram_tensor("x_t", [D, S], F16, kind="ExternalInput").ap()
    mem_t = nc.dram_tensor("mem_t", [D, S], F16, kind="ExternalInput").ap()
    wq_t = nc.dram_tensor("wq_t", [D, OD], F16, kind="ExternalInput").ap()
    wk_t = nc.dram_tensor("wk_t", [D, OD], F16, kind="ExternalInput").ap()
    wv_t = nc.dram_tensor("wv_t", [D, OD], F16, kind="ExternalInput").ap()
    wo_t = nc.dram_tensor("wo_t", [OD, D], F16, kind="ExternalInput").ap()
    bq_s = nc.dram_tensor("bq_s", [P, OD // P], F32, kind="ExternalInput").ap()
    bk_s = nc.dram_tensor("bk_s", [P, OD // P], F32, kind="ExternalInput").ap()
    bo_s = nc.dram_tensor("bo_s", [P, D // P], F32, kind="ExternalInput").ap()
    maskb = nc.dram_tensor("maskb", [P, NKC], F32, kind="ExternalInput").ap()
    out_t = nc.dram_tensor("out_t", [D, S], F32, kind="ExternalOutput").ap()

    x_c = x_t.rearrange("(c p) s -> c p s", p=P)
    m_c = mem_t.rearrange("(c p) s -> c p s", p=P)
    wq_c = wq_t.rearrange("(c p) o -> c p o", p=P)
    wk_c = wk_t.rearrange("(c p) o -> c p o", p=P)
    wv_c = wv_t.rearrange("(c p) o -> c p o", p=P)
    wo_c = wo_t.rearrange("(c p) o -> c p o", p=P)

    NMT = OD // P   # 4 head-pairs

    with tile.TileContext(nc) as tc, ExitStack() as ctx:
        q_pool = ctx.enter_context(tc.tile_pool(name="qt", bufs=1))
        k_pool = ctx.enter_context(tc.tile_pool(name="kt", bufs=1))
        v_pool = ctx.enter_context(tc.tile_pool(name="va", bufs=1))
        a_pool = ctx.enter_context(tc.tile_pool(name="at", bufs=1))
        c_pool = ctx.enter_context(tc.tile_pool(name="cst", bufs=1))
        w_pool = ctx.enter_context(tc.tile_pool(name="wt", bufs=10))
        e_pool = ctx.enter_context(tc.tile_pool(name="es", bufs=6))
        n_pool = ctx.enter_context(tc.tile_pool(name="nrm", bufs=2))
        o_pool = ctx.enter_context(tc.tile_pool(name="ev", bufs=2))
        psum_pool = ctx.enter_context(tc.tile_pool(name="ps", bufs=1, space="PSUM"))
        m_pool = ctx.enter_context(tc.tile_pool(name="mm", bufs=8))
        x_pool = ctx.enter_context(tc.tile_pool(name="xx", bufs=8))

        # ---- constants ----
        bq_sb = c_pool.tile([P, OD // P], F32, tag="bq")
        bk_sb = c_pool.tile([P, OD // P], F32, tag="bk")
        bo_sb = c_pool.tile([P, D // P], F32, tag="bo")
        mk_sb = c_pool.tile([P, NKC], F32, tag="mk")
        nc.sync.dma_start(out=bq_sb[:], in_=bq_s[:])
        nc.sync.dma_start(out=bk_sb[:], in_=bk_s[:])
        nc.sync.dma_start(out=bo_sb[:], in_=bo_s[:])
        nc.sync.dma_start(out=mk_sb[:], in_=maskb[:])
        ones_f = c_pool.tile([P, NH], F32, tag="onef")
        nc.vector.memset(ones_f[:], 1.0)
        ones_r = c_pool.tile([P, NH], F16, tag="oner")
        nc.vector.tensor_copy(ones_r[:], ones_f[:])

        # ---- bulk input DMAs: memory first (K/V-proj), x next (Q-proj) ----
        m_tiles = []
        for i in range(NDC):
            t = m_pool.tile([P, S], F16, tag="m", name="mt")
            eng = nc.sync if i % 2 == 0 else nc.gpsimd
            eng.dma_start(out=t[:], in_=m_c[i])
            m_tiles.append(t)
        x_tiles = []
        for i in range(NDC):
            t = x_pool.tile([P, S], F16, tag="x", name="xt")
            eng = nc.sync if i % 2 == 0 else nc.gpsimd
            eng.dma_start(out=t[:], in_=x_c[i])
            x_tiles.append(t)

        # ---- persistent tiles ----
        qT = [q_pool.tile([P, S], F16, tag=f"q{m}", name=f"q{m}")
              for m in range(NMT)]
        # kT packs a head pair: partitions 0:64 = head 2m, 64:128 = head 2m+1
        kT = [k_pool.tile([P, S], F16, tag=f"k{m}", name=f"k{m}")
              for m in range(NMT)]
        v_aug = [v_pool.tile([P, NH, 65], F16, tag=f"v{st}", name=f"v{st}")
                 for st in range(NKC)]
        attn = [a_pool.tile([P, S], F16, tag=f"a{m}", name=f"a{m}")
                for m in range(NMT)]

        def k_proj(m):
            wk_tiles = []
            for i in range(NDC):
                wt = w_pool.tile([P, P], F16, tag="w", name="wkt", bufs=10)
                nc.sync.dma_start(out=wt[:], in_=wk_c[i, :, m * P:(m + 1) * P])
                wk_tiles.append(wt)
            for n in range(2):
                csl = slice(n * 1024, (n + 1) * 1024)
                ps = psum_pool.tile([P, 1024], F32,
                                    tag="lg_e" if n == 0 else "lg_o", name="psk")
                for i in range(NDC):
                    for j in range(2):
                        nc.tensor.matmul(
                            ps[:, j * 512:(j + 1) * 512], wk_tiles[i][:],
                            m_tiles[i][:, n * 1024 + j * 512:
                                       n * 1024 + (j + 1) * 512],
                            start=(i == 0), stop=(i == NDC - 1),
                        )
                nc.vector.tensor_scalar_add(kT[m][:, csl], ps[:], bk_sb[:, m:m + 1])

        def q_proj(mt):
            wq_tiles = []
            for i in range(NDC):
                wt = w_pool.tile([P, P], F16, tag="w", name="wqt", bufs=10)
                nc.sync.dma_start(out=wt[:], in_=wq_c[i, :, mt * P:(mt + 1) * P])
                wq_tiles.append(wt)
            for n in range(2):
                csl = slice(n * 1024, (n + 1) * 1024)
                ps = psum_pool.tile([P, 1024], F32,
                                    tag="lg_e" if n == 0 else "lg_o", name="psq")
                for i in range(NDC):
                    for j in range(2):
                        nc.tensor.matmul(
                            ps[:, j * 512:(j + 1) * 512], wq_tiles[i][:],
                            x_tiles[i][:, n * 1024 + j * 512:
                                       n * 1024 + (j + 1) * 512],
                            start=(i == 0), stop=(i == NDC - 1),
                        )
                nc.vector.tensor_scalar_add(qT[mt][:, csl], ps[:], bq_sb[:, mt:mt + 1])

        # V-proj weight tiles (loaded once; V itself is emitted just-in-time
        # inside the first attention loop so S/exp work starts immediately)
        wv_tiles = []
        for i in range(NDC):
            wt = w_pool.tile([P, OD], F16, tag="wv", name="wvt", bufs=8)
            nc.sync.dma_start(out=wt[:], in_=wv_c[i])
            wv_tiles.append(wt)

        def v_proj(st):
            ps = psum_pool.tile([P, 1024], F32,
                                tag="lg_e" if st % 2 == 0 else "lg_o", name="psv")
            for i in range(NDC):
                nc.tensor.matmul(
                    ps[:, 0:OD], m_tiles[i][:, st * P:(st + 1) * P],
                    wv_tiles[i][:],
                    start=(i == 0), stop=(i == NDC - 1),
                )
            nc.vector.tensor_copy(
                v_aug[st][:, 0:NH, 0:64],
                ps[:, 0:OD].rearrange("p (h d) -> p h d", h=NH),
            )
            nc.gpsimd.tensor_copy(
                v_aug[st][:, 0:NH, 64:65], ones_r[:].unsqueeze(2))

        # ---- startup: K-proj(0), Q-proj(0) so exp starts ASAP ----
        k_proj(0)
        q_proj(0)

        # ---- attention: head pair mt = heads (2mt, 2mt+1) ----
        for mt in range(NMT):
            for qh in range(2):
                av_e = psum_pool.tile([P, 1024], F32, tag="av_e", name="av_e")
                av_o = psum_pool.tile([P, 1024], F32, tag="av_o", name="av_o")
                for kc in range(NKC):
                    if mt == 0 and qh == 0:
                        v_proj(kc)
                    lg_e = psum_pool.tile([P, 1024], F32, tag="lg_e", name="lg_e")
                    lg_o = psum_pool.tile([P, 1024], F32, tag="lg_o", name="lg_o")
                    for j in range(2):
                        qsl = slice(qh * 1024 + j * 512, qh * 1024 + (j + 1) * 512)
                        nc.tensor.matmul(
                            lg_e[:, j * 512:(j + 1) * 512],
                            kT[mt][0:64, kc * P:(kc + 1) * P],
                            qT[mt][0:64, qsl],
                            start=True, stop=True,
                        )
                    for j in range(2):
                        qsl = slice(qh * 1024 + j * 512, qh * 1024 + (j + 1) * 512)
                        nc.tensor.matmul(
                            lg_o[:, j * 512:(j + 1) * 512],
                            kT[mt][64:128, kc * P:(kc + 1) * P],
                            qT[mt][64:128, qsl],
                            start=True, stop=True,
                        )
                    es_e = e_pool.tile([P, 1024], F16, tag="es")
                    nc.scalar.activation(
                        es_e[:], lg_e[:], AF.Exp,
                        bias=mk_sb[:, kc:kc + 1], scale=0.125)
                    es_o = e_pool.tile([P, 1024], F16, tag="es")
                    nc.scalar.activation(
                        es_o[:], lg_o[:], AF.Exp,
                        bias=mk_sb[:, kc:kc + 1], scale=0.125)
                    va_flat = v_aug[kc][:].rearrange("p h d -> p (h d)")
                    he, ho = 2 * mt, 2 * mt + 1
                    for j in range(2):
                        nc.tensor.matmul(
                            av_e[0:65, j * 512:(j + 1) * 512],
                            va_flat[:, 65 * he:65 * he + 65],
                            es_e[:, j * 512:(j + 1) * 512],
                            start=(kc == 0), stop=(kc == NKC - 1),
                        )
                    for j in range(2):
                        nc.tensor.matmul(
                            av_o[0:65, j * 512:(j + 1) * 512],
                            va_flat[:, 65 * ho:65 * ho + 65],
                            es_o[:, j * 512:(j + 1) * 512],
                            start=(kc == 0), stop=(kc == NKC - 1),
                        )
                q_sl = slice(qh * 1024, (qh + 1) * 1024)
                for ro, av in ((0, av_e), (64, av_o)):
                    r0 = n_pool.tile([1, 1024], F32, tag="r0")
                    bc = n_pool.tile([64, 1024], F32, tag="bc")
                    nc.vector.reciprocal(r0[:], av[64:65, :])
                    nc.gpsimd.partition_broadcast(bc[:], r0[:])
                    nc.vector.tensor_mul(
                        attn[mt][ro:ro + 64, q_sl], av[0:64, :], bc[:])
                # overlap later projections under the ACT-bound window
                if mt == 0 and qh == 0:
                    k_proj(1)
                elif mt == 0 and qh == 1:
                    k_proj(2)
                    q_proj(1)
                elif mt == 1 and qh == 0:
                    k_proj(3)
                elif mt == 1 and qh == 1:
                    q_proj(2)
                elif mt == 2 and qh == 1:
                    q_proj(3)

        # ---- out.T = wo_t.T @ attn.T (+bo_eff) ----
        for m in range(D // P):
            wo_tiles = []
            for i in range(OD // P):
                wt = w_pool.tile([P, P], F16, tag="w", name="wot", bufs=10)
                nc.sync.dma_start(out=wt[:], in_=wo_c[i, :, m * P:(m + 1) * P])
                wo_tiles.append(wt)
            for n in range(2):
                csl = slice(n * 1024, (n + 1) * 1024)
                ps = psum_pool.tile([P, 1024], F32,
                                    tag="av_e" if n == 0 else "av_o", name="pso")
                for i in range(OD // P):
                    for j in range(2):
                        nc.tensor.matmul(
                            ps[:, j * 512:(j + 1) * 512], wo_tiles[i][:],
                            attn[i][:, n * 1024 + j * 512:
                                    n * 1024 + (j + 1) * 512],
                            start=(i == 0), stop=(i == OD // P - 1),
                        )
                ev = o_pool.tile([P, 1024], F32, tag="ev")
                nc.vector.tensor_scalar_add(ev[:], ps[:], bo_sb[:, m:m + 1])
                nc.sync.dma_start(out=out_t[m * P:(m + 1) * P, csl], in_=ev[:])

    nc.compile()
    return nc


def _prep_inputs(x, memory, mask, wq, bq, wk, bk, wv, bv, wo, bo):
    f = np.float32
    h = np.float16
    wqT = np.ascontiguousarray(wq.T, dtype=f)
    wkT = np.ascontiguousarray(wk.T, dtype=f)
    wvT = np.ascontiguousarray(wv.T, dtype=f)
    woT = np.ascontiguousarray(wo.T, dtype=f)
    bo_eff = (bo.astype(f) + wo.astype(f) @ bv.astype(f))
    zeros_bo = np.zeros_like(bo_eff)
    in_maps = []
    for c in range(NCORES):
        b, g = divmod(c, 2)
        sl = slice(g * OD, (g + 1) * OD)
        bo_c = bo_eff if g == 0 else zeros_bo
        in_maps.append({
            "x_t": np.ascontiguousarray(x[b].T, dtype=h),
            "mem_t": np.ascontiguousarray(memory[b].T, dtype=h),
            "wq_t": np.ascontiguousarray(wqT[:, sl]).astype(h),
            "wk_t": np.ascontiguousarray(wkT[:, sl]).astype(h),
            "wv_t": np.ascontiguousarray(wvT[:, sl]).astype(h),
            "wo_t": np.ascontiguousarray(woT[sl, :]).astype(h),
            "bq_s": np.ascontiguousarray(bq[sl].astype(f).reshape(OD // P, P).T),
            "bk_s": np.ascontiguousarray(bk[sl].astype(f).reshape(OD // P, P).T),
            "bo_s": np.ascontiguousarray(bo_c.reshape(D // P, P).T),
            "maskb": np.ascontiguousarray(
                np.where(mask[b], np.float32(NEG), np.float32(0.0))
                .astype(f).reshape(NKC, P).T),
        })
    return in_maps


def kernel(x, memory, mask, wq, bq, wk, bk, wv, bv, wo, bo, **run_kwargs):
    x = np.asarray(x, dtype=np.float32)
    memory = np.asarray(memory, dtype=np.float32)
    mask = np.asarray(mask)
    if "nc" not in _cache:
        _cache["nc"] = _build()
    nc = _cache["nc"]
    in_maps = _prep_inputs(x, memory, mask, wq, bq, wk, bk, wv, bv, wo, bo)
    res = run_bass_kernel_spmd(nc, in_maps, list(range(NCORES)), **run_kwargs)
    out = np.empty((B, S, D), dtype=np.float32)
    for b in range(B):
        part = res.results[2 * b]["out_t"] + res.results[2 * b + 1]["out_t"]
        out[b] = part.T
    if run_kwargs:
        _cache["last_results"] = res
    return out


# revision 5
# speedup vs baseline: 1.1954x; 1.1954x over previous
"""Multi-head cross-attention (B=4, S=2048, D=1024, H=16) on 8 Trainium2 cores.

Sharding: hybrid data/tensor parallel. Core c handles batch b = c//2 and
head-group g = c%2 (8 of the 16 heads, i.e. 512 of the 1024 q/k/v dims).
Each core computes a partial out-projection over its 512 attention dims;
the host sums the two partials per batch.

Design (v2):
- The S=K.T@Q logits matmuls contract over head_dim=64 and are issued as
  PE row-tiled pairs (tile_position (0,0)/(64,0)): even head on array
  rows 0:63, odd head on rows 64:127, streaming concurrently. kT packs a
  head pair per tile.
- The ACT engine runs ONLY exp (its ~266us is the kernel floor). The
  key-padding mask is applied by zeroing masked keys' V rows and ones
  column (exactly equivalent to -inf logits), so exp needs no per-chunk
  bias and one exp spans two key chunks ([128,1024]).
- All projection work (K1-3, Q1-3, O) is emitted as small filler batches
  inside the ACT-bound attention loop so the PE never idles (keeps the
  HAM clock-gate at 2.4GHz and hides projection time entirely).
- PSUM: lg_e(2) lg_o(2) av_e(1) av_o(1) pj(2) = 8 banks.

Per-core math (fp16 PE fast path):
  K.T = wk_t.T @ mem_t (+bk), V = mem_t.T @ wv_t (masked, ones column),
  Q.T = wq_t.T @ x_t (+bq)
  per head pair, query block qb (512), key chunk pair:
    S.T = K_h @ Q_h.T (row-tiled pair), P.T = exp(0.125*S.T)
    AV.T += [V|1].T @ P.T   [65, 512] PSUM accum
  attn.T = AV.T[0:64] * recip(AV.T[64])
  out.T = wo_t.T @ attn.T (+bo_eff on core g=0), host sums core pairs.

bv is folded into bo on the host (softmax rows sum to 1).
"""

import numpy as np

import concourse.bacc as bacc
import concourse.mybir as mybir
from concourse import tile
from concourse.bass_utils import run_bass_kernel_spmd

F32 = mybir.dt.float32
F16 = mybir.dt.float16
AF = mybir.ActivationFunctionType

B, S, D = 4, 2048, 1024
H, HD = 16, 64
NCORES = 8
NH = 8          # heads per core
OD = NH * HD    # 512 attention dims per core
P = 128
NDC = D // P    # 8 d-chunks
NKC = S // P    # 16 key chunks
NMT = OD // P   # 4 head-pairs

_cache = {}


def _build():
    from contextlib import ExitStack

    nc = bacc.Bacc(None, target_bir_lowering=False, debug=False)

    x_t = nc.dram_tensor("x_t", [D, S], F16, kind="ExternalInput").ap()
    mem_t = nc.dram_tensor("mem_t", [D, S], F16, kind="ExternalInput").ap()
    wq_t = nc.dram_tensor("wq_t", [D, OD], F16, kind="ExternalInput").ap()
    wk_t = nc.dram_tensor("wk_t", [D, OD], F16, kind="ExternalInput").ap()
    wv_t = nc.dram_tensor("wv_t", [D, OD], F16, kind="ExternalInput").ap()
    wo_t = nc.dram_tensor("wo_t", [OD, D], F16, kind="ExternalInput").ap()
    bq_s = nc.dram_tensor("bq_s", [P, OD // P], F32, kind="ExternalInput").ap()
    bk_s = nc.dram_tensor("bk_s", [P, OD // P], F32, kind="ExternalInput").ap()
    bo_s = nc.dram_tensor("bo_s", [P, D // P], F32, kind="ExternalInput").ap()
    vmask = nc.dram_tensor("vmask", [P, NKC], F32, kind="ExternalInput").ap()
    vmask8 = nc.dram_tensor("vmask8", [P, NKC * NH], F16,
                            kind="ExternalInput").ap()
    out_t = nc.dram_tensor("out_t", [D, S], F32, kind="ExternalOutput").ap()

    x_c = x_t.rearrange("(c p) s -> c p s", p=P)
    m_c = mem_t.rearrange("(c p) s -> c p s", p=P)
    wq_c = wq_t.rearrange("(c p) o -> c p o", p=P)
    wk_c = wk_t.rearrange("(c p) o -> c p o", p=P)
    wv_c = wv_t.rearrange("(c p) o -> c p o", p=P)
    wo_c = wo_t.rearrange("(c p) o -> c p o", p=P)

    with tile.TileContext(nc) as tc, ExitStack() as ctx:
        q_pool = ctx.enter_context(tc.tile_pool(name="qt", bufs=1))
        k_pool = ctx.enter_context(tc.tile_pool(name="kt", bufs=1))
        v_pool = ctx.enter_context(tc.tile_pool(name="va", bufs=1))
        a_pool = ctx.enter_context(tc.tile_pool(name="at", bufs=1))
        c_pool = ctx.enter_context(tc.tile_pool(name="cst", bufs=1))
        w_pool = ctx.enter_context(tc.tile_pool(name="wt", bufs=10))
        e_pool = ctx.enter_context(tc.tile_pool(name="es", bufs=6))
        n_pool = ctx.enter_context(tc.tile_pool(name="nrm", bufs=2))
        o_pool = ctx.enter_context(tc.tile_pool(name="ev", bufs=3))
        psum_pool = ctx.enter_context(tc.tile_pool(name="ps", bufs=1, space="PSUM"))
        m_pool = ctx.enter_context(tc.tile_pool(name="mm", bufs=8))
        x_pool = ctx.enter_context(tc.tile_pool(name="xx", bufs=8))

        # ---- constants ----
        bq_sb = c_pool.tile([P, OD // P], F32, tag="bq")
        bk_sb = c_pool.tile([P, OD // P], F32, tag="bk")
        bo_sb = c_pool.tile([P, D // P], F32, tag="bo")
        vm_sb = c_pool.tile([P, NKC], F32, tag="vm")
        vm8_sb = c_pool.tile([P, NKC, NH], F16, tag="vm8")
        nc.sync.dma_start(out=bq_sb[:], in_=bq_s[:])
        nc.sync.dma_start(out=bk_sb[:], in_=bk_s[:])
        nc.sync.dma_start(out=bo_sb[:], in_=bo_s[:])
        nc.sync.dma_start(out=vm_sb[:], in_=vmask[:])
        nc.sync.dma_start(
            out=vm8_sb[:], in_=vmask8.rearrange("p (s h) -> p s h", h=NH))

        # ---- bulk input DMAs: memory first (K/V-proj), x next (Q-proj) ----
        m_tiles = []
        for i in range(NDC):
            t = m_pool.tile([P, S], F16, tag="m", name="mt")
            eng = nc.sync if i % 2 == 0 else nc.gpsimd
            eng.dma_start(out=t[:], in_=m_c[i])
            m_tiles.append(t)
        x_tiles = []
        for i in range(NDC):
            t = x_pool.tile([P, S], F16, tag="x", name="xt")
            eng = nc.sync if i % 2 == 0 else nc.gpsimd
            eng.dma_start(out=t[:], in_=x_c[i])
            x_tiles.append(t)

        # ---- persistent tiles ----
        qT = [q_pool.tile([P, S], F16, tag=f"q{m}", name=f"q{m}")
              for m in range(NMT)]
        # kT packs a head pair: partitions 0:64 = head 2m, 64:128 = head 2m+1
        kT = [k_pool.tile([P, S], F16, tag=f"k{m}", name=f"k{m}")
              for m in range(NMT)]
        v_aug = [v_pool.tile([P, NH, 65], F16, tag=f"v{st}", name=f"v{st}")
                 for st in range(NKC)]
        attn = [a_pool.tile([P, S], F16, tag=f"a{m}", name=f"a{m}")
                for m in range(NMT)]

        # ---- projection emitters (as lists of small closures) ----
        def kq_proj_steps(wc, src_tiles, dst, bias, m):
            """K/Q projection chunk m -> dst[:, :]: 4x(8 matmuls + evac)."""
            w_tiles = []

            def load_w():
                for i in range(NDC):
                    wt = w_pool.tile([P, P], F16, tag="w", name="wkq", bufs=10)
                    nc.sync.dma_start(out=wt[:], in_=wc[i, :, m * P:(m + 1) * P])
                    w_tiles.append(wt)
            steps = [load_w]
            for half in range(4):
                csl = slice(half * 512, (half + 1) * 512)
                ps = []

                def mm(i, ps=ps, csl=csl):
                    if i == 0:
                        ps.append(psum_pool.tile([P, 512], F32, tag="pj",
                                                 name="pskq", bufs=2))
                    nc.tensor.matmul(
                        ps[0][:], w_tiles[i][:], src_tiles[i][:, csl],
                        start=(i == 0), stop=(i == NDC - 1))
                for i in range(NDC):
                    steps.append(lambda i=i, mm=mm: mm(i))

                def evac(ps=ps, csl=csl):
                    nc.vector.tensor_scalar_add(
                        dst[:, csl], ps[0][:], bias[:, m:m + 1])
                steps.append(evac)
            return steps

        # V-proj weight tiles + per-token-chunk V projection
        wv_tiles = []
        for i in range(NDC):
            wt = w_pool.tile([P, OD], F16, tag="wv", name="wvt", bufs=8)
            nc.sync.dma_start(out=wt[:], in_=wv_c[i])
            wv_tiles.append(wt)

        def v_proj(st):
            ps = psum_pool.tile([P, 512], F32, tag="pj", name="psv", bufs=2)
            for i in range(NDC):
                nc.tensor.matmul(
                    ps[:], m_tiles[i][:, st * P:(st + 1) * P], wv_tiles[i][:],
                    start=(i == 0), stop=(i == NDC - 1))
            nc.vector.tensor_scalar_mul(
                v_aug[st][:, 0:NH, 0:64],
                ps[:].rearrange("p (h d) -> p h d", h=NH),
                vm_sb[:, st:st + 1])
            nc.gpsimd.tensor_copy(
                v_aug[st][:, 0:NH, 64:65], vm8_sb[:, st, :].unsqueeze(2))

        def o_proj_steps(m, jb):
            """out chunk [m*128:(m+1)*128, jb*512:(jb+1)*512]."""
            ps = []

            def mm(i):
                if i == 0:
                    ps.append(psum_pool.tile([P, 512], F32, tag="pj",
                                             name="pso", bufs=2))
                nc.tensor.matmul(
                    ps[0][:], wo_tiles[m][i][:],
                    attn[i][:, jb * 512:(jb + 1) * 512],
                    start=(i == 0), stop=(i == NMT - 1))
            steps = [lambda i=i, mm=mm: mm(i) for i in range(NMT)]

            def evac():
                ev = o_pool.tile([P, 512], F32, tag="ev")
                nc.vector.tensor_scalar_add(ev[:], ps[0][:], bo_sb[:, m:m + 1])
                nc.sync.dma_start(
                    out=out_t[m * P:(m + 1) * P, jb * 512:(jb + 1) * 512],
                    in_=ev[:])
            steps.append(evac)
            return steps

        # ---- startup: K0; then V interleaved with Q0 so exp starts ASAP ----
        for step in kq_proj_steps(wk_c, m_tiles, kT[0], bk_sb, 0):
            step()
        q0_steps = kq_proj_steps(wq_c, x_tiles, qT[0], bq_sb, 0)
        q0_steps[0]()           # weight DMA
        q0_steps = q0_steps[1:]
        qi = 0
        for st in range(NKC):
            v_proj(st)
            take = 3 if st < 12 else len(q0_steps) - qi
            for _ in range(max(0, take)):
                if qi < len(q0_steps):
                    q0_steps[qi]()
                    qi += 1

        # O-proj weight tiles, loaded late (registered here, DMA'd in fills)
        wo_tiles = [[None] * NMT for _ in range(D // P)]

        def load_wo(m):
            for i in range(NMT):
                wt = w_pool.tile([P, P], F16, tag="wo", name="wot", bufs=32)
                nc.sync.dma_start(out=wt[:], in_=wo_c[i, :, m * P:(m + 1) * P])
                wo_tiles[m][i] = wt

        # ---- fill stream: projections hidden inside the attention loop ----
        fills = []
        fills += kq_proj_steps(wk_c, m_tiles, kT[1], bk_sb, 1)
        fills += kq_proj_steps(wq_c, x_tiles, qT[1], bq_sb, 1)
        fills += kq_proj_steps(wk_c, m_tiles, kT[2], bk_sb, 2)
        fills += kq_proj_steps(wq_c, x_tiles, qT[2], bq_sb, 2)
        fills += kq_proj_steps(wk_c, m_tiles, kT[3], bk_sb, 3)
        fills += kq_proj_steps(wq_c, x_tiles, qT[3], bq_sb, 3)
        n_kq = len(fills)
        fi = 0

        def budget(it):
            # iters 0..7: warmup, none. 8..31: K1+Q1 (must finish by mt1).
            # 32..95: K2,Q2,K3,Q3. 104+: O-proj for ready query columns.
            if it < 8:
                return 0
            if it < 32:
                return 4
            if it < 96:
                return 3
            return 5

        # ---- attention: head pair mt, query block qb (512), kc pair ----
        it = 0
        for mt in range(NMT):
            he, ho = 2 * mt, 2 * mt + 1
            for qb in range(4):
                qsl = slice(qb * 512, (qb + 1) * 512)
                av_e = psum_pool.tile([P, 512], F32, tag="av_e", name="av_e")
                av_o = psum_pool.tile([P, 512], F32, tag="av_o", name="av_o")
                for k2 in range(NKC // 2):
                    ka, kb = 2 * k2, 2 * k2 + 1
                    lg_e = psum_pool.tile([P, 1024], F32, tag="lg_e", name="lg_e")
                    lg_o = psum_pool.tile([P, 1024], F32, tag="lg_o", name="lg_o")
                    for half, kc in ((0, ka), (1, kb)):
                        nc.tensor.matmul(
                            lg_e[:, half * 512:(half + 1) * 512],
                            kT[mt][0:64, kc * P:(kc + 1) * P],
                            qT[mt][0:64, qsl], start=True, stop=True)
                    for half, kc in ((0, ka), (1, kb)):
                        nc.tensor.matmul(
                            lg_o[:, half * 512:(half + 1) * 512],
                            kT[mt][64:128, kc * P:(kc + 1) * P],
                            qT[mt][64:128, qsl], start=True, stop=True)
                    es_e = e_pool.tile([P, 1024], F16, tag="es")
                    nc.scalar.activation(es_e[:], lg_e[:], AF.Exp, scale=0.125)
                    es_o = e_pool.tile([P, 1024], F16, tag="es")
                    nc.scalar.activation(es_o[:], lg_o[:], AF.Exp, scale=0.125)
                    for half, kc in ((0, ka), (1, kb)):
                        va = v_aug[kc][:].rearrange("p h d -> p (h d)")
                        nc.tensor.matmul(
                            av_e[0:65, :], va[:, 65 * he:65 * he + 65],
                            es_e[:, half * 512:(half + 1) * 512],
                            start=(k2 == 0 and half == 0),
                            stop=(k2 == NKC // 2 - 1 and half == 1))
                    for half, kc in ((0, ka), (1, kb)):
                        va = v_aug[kc][:].rearrange("p h d -> p (h d)")
                        nc.tensor.matmul(
                            av_o[0:65, :], va[:, 65 * ho:65 * ho + 65],
                            es_o[:, half * 512:(half + 1) * 512],
                            start=(k2 == 0 and half == 0),
                            stop=(k2 == NKC // 2 - 1 and half == 1))
                    # filler work keeps the PE warm and hides projections
                    if it == 96:
                        for m in range(D // P):
                            load_wo(m)
                    nb = budget(it)
                    while nb > 0 and fi < len(fills):
                        fills[fi]()
                        fi += 1
                        nb -= 1
                    if it == 103:
                        # register O-proj fills for ready columns qb0, qb1
                        for jb in range(2):
                            for m in range(D // P):
                                fills += o_proj_steps(m, jb)
                    it += 1
                for ro, av in ((0, av_e), (64, av_o)):
                    r0 = n_pool.tile([1, 512], F32, tag="r0")
                    bc = n_pool.tile([64, 512], F32, tag="bc")
                    nc.vector.reciprocal(r0[:], av[64:65, :])
                    nc.gpsimd.partition_broadcast(bc[:], r0[:])
                    nc.vector.tensor_mul(
                        attn[mt][ro:ro + 64, qsl], av[0:64, :], bc[:])

        # ---- remaining fills + O-proj for columns qb2, qb3 ----
        while fi < len(fills):
            fills[fi]()
            fi += 1
        for jb in range(2, 4):
            for m in range(D // P):
                for step in o_proj_steps(m, jb):
                    step()

    nc.compile()
    return nc


def _prep_inputs(x, memory, mask, wq, bq, wk, bk, wv, bv, wo, bo):
    f = np.float32
    h = np.float16
    wqT = np.ascontiguousarray(wq.T, dtype=f)
    wkT = np.ascontiguousarray(wk.T, dtype=f)
    wvT = np.ascontiguousarray(wv.T, dtype=f)
    woT = np.ascontiguousarray(wo.T, dtype=f)
    bo_eff = (bo.astype(f) + wo.astype(f) @ bv.astype(f))
    zeros_bo = np.zeros_like(bo_eff)
    in_maps = []
    for c in range(NCORES):
        b, g = divmod(c, 2)
        sl = slice(g * OD, (g + 1) * OD)
        bo_c = bo_eff if g == 0 else zeros_bo
        vm = np.where(mask[b], np.float32(0.0), np.float32(1.0)).astype(f)
        vm_s = np.ascontiguousarray(vm.reshape(NKC, P).T)      # [P, NKC]
        vm8 = np.repeat(vm_s.astype(h)[:, :, None], NH, axis=2)  # [P,NKC,NH]
        in_maps.append({
            "x_t": np.ascontiguousarray(x[b].T, dtype=h),
            "mem_t": np.ascontiguousarray(memory[b].T, dtype=h),
            "wq_t": np.ascontiguousarray(wqT[:, sl]).astype(h),
            "wk_t": np.ascontiguousarray(wkT[:, sl]).astype(h),
            "wv_t": np.ascontiguousarray(wvT[:, sl]).astype(h),
            "wo_t": np.ascontiguousarray(woT[sl, :]).astype(h),
            "bq_s": np.ascontiguousarray(bq[sl].astype(f).reshape(OD // P, P).T),
            "bk_s": np.ascontiguousarray(bk[sl].astype(f).reshape(OD // P, P).T),
            "bo_s": np.ascontiguousarray(bo_c.reshape(D // P, P).T),
            "vmask": vm_s,
            "vmask8": np.ascontiguousarray(vm8.reshape(P, NKC * NH)),
        })
    return in_maps


def kernel(x, memory, mask, wq, bq, wk, bk, wv, bv, wo, bo, **run_kwargs):
    x = np.asarray(x, dtype=np.float32)
    memory = np.asarray(memory, dtype=np.float32)
    mask = np.asarray(mask)
    if "nc" not in _cache:
        _cache["nc"] = _build()
    nc = _cache["nc"]
    in_maps = _prep_inputs(x, memory, mask, wq, bq, wk, bk, wv, bv, wo, bo)
    res = run_bass_kernel_spmd(nc, in_maps, list(range(NCORES)), **run_kwargs)
    out = np.empty((B, S, D), dtype=np.float32)
    for b in range(B):
        part = res.results[2 * b]["out_t"] + res.results[2 * b + 1]["out_t"]
        out[b] = part.T
    if run_kwargs:
        _cache["last_results"] = res
    return out
